# revision 12
# baseline (speedup 1.0000x reference)
"""MultiHeadSelfAttentionWithLagBias on 8 TRN2 NeuronCores.

Sharding: tensor-parallel over heads — 16 heads / 8 cores = 2 heads per
core. Each core computes QKV projections for its head slice (full x),
attention with the lag bias for its 2 heads over both batch elements,
and a partial output projection (its 128 rows of wo). Host sums the 8
partials and adds bo.

v3 (vs the ~247us v2): the out-projection no longer runs as a separate
end phase (which traced at ~60us with PE half-clock-throttled and
<50% busy). Attention is reordered qc-outer / batch-middle / j-inner so
each lag-bias tile is fetched once per q-chunk and served to both
batches; as soon as a (qc, b) pair of O accumulators drains, its
normalize + out-projection matmuls and drains are queued as pending
thunks and dribbled into the attention instruction stream (one per
iteration) so they fill PE/DVE slack under the ACT-bound exp loop
instead of serializing after it. Out-proj PSUM comes from the score
pool's own tag (same [128,1024] f32 shape) so the 8-bank budget
(scores 2x2 + O accums 4x1) is unchanged. PSUM->SBUF drains for O, den
rows, V tiles, and out-proj results run on the otherwise-idle GpSimd
engine. exp(lag_bias) is host-tiled [ji-major, head-minor] so the bias
multiply is a single plain-2D contiguous bf16 tensor_tensor per
iteration. Denominators (ones-column of the PV matmul, row 64) go to a
small DEN tile; normalize broadcasts 1/den via the K=1 ones-row matmul
trick into a score-tag PSUM tile.
"""

import ml_dtypes
import numpy as np
from contextlib import ExitStack

import concourse.bass as bass
import concourse.bacc as bacc
import concourse.mybir as mybir
import concourse.tile as tile
from concourse.bass_utils import run_bass_kernel_spmd
from concourse.masks import make_identity

F32 = mybir.dt.float32
BF16 = mybir.dt.bfloat16
AF = mybir.ActivationFunctionType

N_CORES = 8
B, S, D = 2, 2048, 1024
H, DK = 16, 64
TOK = B * S              # 4096
NQ = 512                 # q-chunk (matmul free dim / PSUM bank)
NQC = S // NQ            # 4 q-chunks per batch
NJ = S // 128            # 16 k-chunks per batch
DCH = D // 128           # 8 contraction chunks

# Set by test.py for profiling; harness leaves these untouched.
TRACE = False
TRACE_DIR = None

_CACHED_NC = None


def _body(ctx: ExitStack, tc, aps):
    nc = tc.nc
    xT, wq, wk, wv, bq, bk, bv, wo, EB, out = (
        aps["xT"], aps["wq"], aps["wk"], aps["wv"], aps["bq"], aps["bk"],
        aps["bv"], aps["wo"], aps["EB"], aps["out"])

    const = ctx.enter_context(tc.tile_pool(name="const", bufs=1))
    persist = ctx.enter_context(tc.tile_pool(name="persist", bufs=1))

    # ---- constants ----
    w_sb = {}
    for name, ap in (("q", wq), ("k", wk), ("v", wv)):
        t = const.tile([128, DCH, 128], BF16, tag=f"w{name}")
        nc.sync.dma_start(t[:], ap.rearrange("(c p) m -> p c m", p=128))
        w_sb[name] = t
    b_sb = {}
    for name, ap in (("q", bq), ("k", bk), ("v", bv)):
        t = const.tile([128, 1], F32, tag=f"b{name}")
        nc.sync.dma_start(t[:], ap[:])
        b_sb[name] = t
    ident = const.tile([128, 128], BF16, tag="id")
    make_identity(nc, ident[:])
    # tiny dummy exp issued during the projection phase so the ~2.7us ACT
    # table load happens off the attention critical path
    escr = const.tile([1, 8], F32, tag="escr")
    nc.vector.memset(escr[:], 0.0)
    # stationary row of ones at partition 64 for broadcasting the softmax
    # denominator (bf16 to match the DEN moving dtype; partition 64 to
    # match the den row's base partition)
    ones_row = const.tile([65, 64], BF16, tag="ones_row")
    nc.vector.memset(ones_row[64:65, :], 1.0)

    # ---- persistent activations ----
    QT = persist.tile([128, TOK], BF16, tag="QT")
    KT = persist.tile([128, TOK], BF16, tag="KT")
    Vb = persist.tile([128, TOK // 128, 130], BF16, tag="Vb")
    # both heads merged on the partition axis (h0 rows 0-63, h1 rows
    # 64-127) so the out-projection contracts K=128 in a single matmul
    OTm = persist.tile([128, TOK], BF16, tag="OTm")
    # softmax denominators on row 64 (base partition matches the O
    # accumulators' ones-row and the ones_row stationary)
    DEN = persist.tile([65, 2, TOK], BF16, tag="DEN")
    # (bf16 den keeps the K=1 broadcast matmul on the bf16 FWL path; the
    # ~0.4% den quantization matches the baseline's OT bf16 den row)

    # ones columns of V_ext (positions 64 and 129 of each 130-stripe);
    # staged via an f32 memset + copy (memset on strided bf16 is
    # unreliable).
    ones_f32 = const.tile([128, 64], F32, tag="ones_f32")
    nc.vector.memset(ones_f32[:], 1.0)
    nc.vector.tensor_copy(
        Vb[:].rearrange("p t (g x) -> p t g x", g=2)[:, :, :, 64:65],
        ones_f32[:].rearrange("p (t g x) -> p t g x", t=TOK // 128, g=2))

    # bias tiles are prefetched from inside the projection loop, so the
    # pool opens early. EB row-block g=(qc*4+jq)*128 holds, for both
    # heads, the [128 k, ji, h, 512 q] exp-bias slab: one contiguous
    # 8KB-per-partition DMA per group.
    ebpool = ctx.enter_context(tc.tile_pool(name="eb", bufs=6))
    ebt_tiles = [None] * 16

    def issue_eb(g):
        t = ebpool.tile([128, 4, 2 * NQ], BF16, tag="eb")
        r = g * 128
        nc.sync.dma_start(
            t[:], EB[r:r + 128, :].rearrange("p (i q) -> p i q", i=4))
        ebt_tiles[g] = t

    # ---- phase 1: QKV projections + V transpose (scoped pools) ----
    with tc.tile_pool(name="xin", bufs=3) as xpool, \
         tc.tile_pool(name="vtp", bufs=1) as vtpool, \
         tc.tile_pool(name="pj", bufs=3, space="PSUM") as pjpool, \
         tc.tile_pool(name="pt", bufs=2, space="PSUM") as ptpool:
        VT = vtpool.tile([128, TOK], BF16, tag="VT")
        xT_r = xT.rearrange("(c p) n -> p c n", p=128)
        # x chunks stream in on the ACT engine's hwdge queue so they are
        # not serialized behind the weight/bias/eb issues on sync (the
        # sync queue alone put the first x chunk ~8us late)
        xts = []
        for t in range(TOK // NQ):
            xts.append(xpool.tile([128, DCH, NQ], BF16, tag="x",
                                  name=f"xt{t}"))
        nc.scalar.dma_start(xts[0][:, 0:2, :], xT_r[:, 0:2, 0:NQ])
        nc.scalar.dma_start(xts[0][:, 2:DCH, :], xT_r[:, 2:DCH, 0:NQ])
        nc.scalar.dma_start(xts[1][:], xT_r[:, :, NQ:2 * NQ])
        # warm the PE (HAM un-throttle needs ~3.4us of sustained matmul
        # activity) during the otherwise-idle startup DMA window
        wps = pjpool.tile([128, 128], F32, tag="warm")
        for w in range(32):
            nc.tensor.matmul(wps[:], ident[:], ident[:],
                             start=True, stop=True)
        for t in range(TOK // NQ):
            xt = xts[t]
            if t + 2 < TOK // NQ:
                nc.scalar.dma_start(
                    xts[t + 2][:], xT_r[:, :, (t + 2) * NQ:(t + 3) * NQ])
            for name, dst in (("q", QT), ("k", KT), ("v", VT)):
                ps = pjpool.tile([128, NQ], F32, tag="pj")
                for d in range(DCH):
                    nc.tensor.matmul(ps[:], w_sb[name][:, d, :], xt[:, d, :],
                                     start=(d == 0), stop=(d == DCH - 1))
                nc.vector.tensor_scalar_add(
                    dst[:, t * NQ:(t + 1) * NQ], ps[:], b_sb[name][:])
            # V transpose for this token chunk (4 x 128-tok tiles); bf16
            # transposes run at 1 cycle/row (vs 2 for f32)
            for u in range(t * 4, t * 4 + 4):
                pt = ptpool.tile([128, 128], BF16, tag="pt")
                nc.tensor.transpose(pt[:], VT[:, u * 128:(u + 1) * 128],
                                    ident[:])
                nc.vector.tensor_copy(
                    Vb[:, u, :].rearrange("p (g x) -> p g x", g=2)[:, :, 0:64],
                    pt[:].rearrange("p (g x) -> p g x", g=2))
            if t == 1:
                nc.scalar.activation(escr[:], escr[:], AF.Exp)
            if t >= 3:
                issue_eb(t - 3)  # prefetch qc0's bias tiles (+ qc1 jq0)

    # wo loaded here, off the startup critical path (fires during attention)
    wo_m = const.tile([128, D], BF16, tag="wo_m")
    nc.sync.dma_start(wo_m[:], wo[:])

    # ---- phase 2: attention (ACT-bound pipeline) ----
    with tc.tile_pool(name="pr", bufs=3) as prpool, \
         tc.tile_pool(name="pe", bufs=3) as pepool, \
         tc.tile_pool(name="sp", bufs=2, space="PSUM") as spool, \
         tc.tile_pool(name="op", bufs=4, space="PSUM") as opool:

        # Boundary drains (den rows + un-normalized O values) are queued
        # as thunks and dribbled one per iteration into the next (qc, b)
        # step so they never head-block the DVE behind a fresh dependency.
        pending = []

        def pop_one():
            if pending:
                pending.pop(0)()

        def queue_drains(qc, b, O_ps):
            q0 = b * S + qc * NQ
            sl = slice(q0, q0 + NQ)
            for hh in range(2):
                def den_copy(h=hh):
                    nc.vector.tensor_copy(DEN[64:65, h, sl],
                                          O_ps[h][64:65, :])
                def o_copy(h=hh):
                    nc.vector.tensor_copy(OTm[h * 64:(h + 1) * 64, sl],
                                          O_ps[h][0:64, :])
                pending.append(den_copy)
                pending.append(o_copy)

        for qc in range(NQC):
            for b in range(2):
                O_ps = [opool.tile([65, NQ], F32, tag="O", name=f"O{h}")
                        for h in range(2)]
                q0 = b * S + qc * NQ
                for jq in range(4):
                    ebt = ebt_tiles[qc * 4 + jq]
                    for ji in range(4):
                        j = jq * 4 + ji
                        k0 = b * S + j * 128
                        sps = spool.tile([128, 2 * NQ], F32, tag="s")
                        for hh in range(2):
                            nc.tensor.matmul(
                                sps[:, hh * NQ:(hh + 1) * NQ],
                                KT[64 * hh:64 * hh + 64, k0:k0 + 128],
                                QT[64 * hh:64 * hh + 64, q0:q0 + NQ],
                                start=True, stop=True)
                        pr = prpool.tile([128, 2 * NQ], BF16, tag="pr")
                        nc.scalar.activation(pr[:], sps[:], AF.Exp)
                        pe = pepool.tile([128, 2 * NQ], BF16, tag="pe")
                        nc.vector.tensor_mul(pe[:], pr[:], ebt[:, ji, :])
                        pop_one()
                        for hh in range(2):
                            nc.tensor.matmul(
                                O_ps[hh][:],
                                Vb[:, b * NJ + j, 65 * hh:65 * hh + 65],
                                pe[:, hh * NQ:(hh + 1) * NQ],
                                start=(j == 0), stop=(j == NJ - 1))
                    if b == 1 and qc < NQC - 1 and not (qc == 0 and jq == 0):
                        # refill this jq's bias slot for the next q-chunk
                        # (g4 = qc1/jq0 was already prefetched in proj)
                        issue_eb((qc + 1) * 4 + jq)
                queue_drains(qc, b, O_ps)
        while pending:
            pending.pop(0)()

    # ---- phase 3: normalize + output projection (deep-pipelined tail) ----
    with tc.tile_pool(name="rp", bufs=2, space="PSUM") as rpool, \
         tc.tile_pool(name="rb", bufs=2) as rbpool, \
         tc.tile_pool(name="os", bufs=3, space="PSUM") as ospool, \
         tc.tile_pool(name="dr", bufs=4) as drpool:
        steps = [(qc, b) for b in range(2) for qc in range(NQC)]

        def normalize(step):
            # broadcast both f32 den rows onto the two partition halves of
            # one PSUM bank via K=1 matmuls, take the fast reciprocal
            # across all 128 partitions in one op, then scale the merged
            # OTm slab in place with a single full-width mul.
            qc, b = step
            sl = slice(b * S + qc * NQ, b * S + qc * NQ + NQ)
            R = rpool.tile([128, NQ], F32, tag="R")
            nc.tensor.matmul(R[0:64, :], ones_row[64:65, :],
                             DEN[64:65, 0, sl], start=True, stop=True)
            nc.tensor.matmul(R[64:128, :], ones_row[64:65, :],
                             DEN[64:65, 1, sl], start=True, stop=True,
                             skip_group_check=True)
            rb = rbpool.tile([128, NQ], F32, tag="rb")
            nc.vector.reciprocal_approx_fast(rb[:], R[:])
            # the in-place scale runs on GpSimd (SBUF-only op) so both
            # copy engines stay free for the PSUM drains below
            nc.gpsimd.tensor_mul(OTm[:, sl], OTm[:, sl], rb[:])

        normalize(steps[0])
        normalize(steps[1])
        for i, (qc, b) in enumerate(steps):
            if i + 2 < len(steps):
                normalize(steps[i + 2])
            u0 = (b * S + qc * NQ) // 128
            for u in range(u0, u0 + 4):
                ops = ospool.tile([128, 2 * NQ], F32, tag="os")
                for half in range(2):
                    osl = slice(half * NQ, (half + 1) * NQ)
                    nc.tensor.matmul(ops[:, osl],
                                     OTm[:, u * 128:(u + 1) * 128],
                                     wo_m[:, osl], start=True, stop=True)
                osb = drpool.tile([128, 2 * NQ], BF16, tag="dr")
                # alternate whole-tile drains between the two PSUM-capable
                # engines (ACT slightly favored; DVE also owns the recips)
                if u % 8 < 5:
                    nc.scalar.copy(osb[:], ops[:])
                else:
                    nc.vector.tensor_copy(osb[:], ops[:])
                nc.sync.dma_start(out[u * 128:(u + 1) * 128, :], osb[:])


def build_program():
    nc = bacc.Bacc("TRN2", target_bir_lowering=False, debug=False,
                   enable_asserts=False, num_devices=N_CORES)
    aps = {}
    specs = [
        ("xT", (D, TOK), BF16), ("wq", (D, 128), BF16), ("wk", (D, 128), BF16),
        ("wv", (D, 128), BF16), ("bq", (128, 1), F32), ("bk", (128, 1), F32),
        ("bv", (128, 1), F32), ("wo", (128, D), BF16),
        ("EB", (NQC * 4 * 128, 2 * 2048), BF16),
    ]
    for name, shape, dt in specs:
        aps[name] = nc.dram_tensor(name, shape, dt, kind="ExternalInput").ap()
    aps["out"] = nc.dram_tensor("out", (TOK, D), BF16,
                                kind="ExternalOutput").ap()
    with tile.TileContext(nc) as tc:
        with ExitStack() as ctx:
            _body(ctx, tc, aps)
    nc.compile()
    return nc


def _get_nc():
    global _CACHED_NC
    if _CACHED_NC is None:
        _CACHED_NC = build_program()
    return _CACHED_NC


def _host_prep(x, lag, wq, bq, wk, bk, wv, bv, wo, bo, lag_bias):
    x = np.asarray(x, dtype=np.float32)
    lag = np.asarray(lag).astype(np.int64)
    xT = np.ascontiguousarray(
        x.reshape(TOK, D).T.astype(ml_dtypes.bfloat16))
    ld = np.abs(lag[:, None] - lag[None, :]).astype(np.int64)
    lag_bias = np.asarray(lag_bias, dtype=np.float32)
    exp_lb = np.exp(lag_bias).astype(np.float32)
    scale = np.float32(1.0 / np.sqrt(DK))
    wq = np.asarray(wq, dtype=np.float32) * scale
    bq = np.asarray(bq, dtype=np.float32) * scale
    in_maps = []
    for c in range(N_CORES):
        sl = slice(c * 128, (c + 1) * 128)
        cm = {
            "xT": xT,
            "wq": np.ascontiguousarray(wq[:, sl].astype(ml_dtypes.bfloat16)),
            "wk": np.ascontiguousarray(
                np.asarray(wk, np.float32)[:, sl].astype(ml_dtypes.bfloat16)),
            "wv": np.ascontiguousarray(
                np.asarray(wv, np.float32)[:, sl].astype(ml_dtypes.bfloat16)),
            "bq": np.ascontiguousarray(bq[sl].reshape(128, 1)),
            "bk": np.ascontiguousarray(
                np.asarray(bk, np.float32)[sl].reshape(128, 1)),
            "bv": np.ascontiguousarray(
                np.asarray(bv, np.float32)[sl].reshape(128, 1)),
            "wo": np.ascontiguousarray(
                np.asarray(wo, np.float32)[sl, :].astype(ml_dtypes.bfloat16)),
        }
        # exp(bias) for both heads gathered, then pre-tiled so each
        # (qc, jq) DMA reads [128, 8KB-contiguous-per-partition]:
        #   row (qc*4+jq)*128 + p, col ji*1024 + h*512 + q
        #   maps to bias[h, k = (jq*4+ji)*128 + p, qpos = qc*512 + q]
        eb = exp_lb[2 * c:2 * c + 2][:, ld]                  # (2, S_k, S_q)
        # (h, (jq ji p), (qc q)) -> (qc, jq, p, ji, h, q)
        eb7 = eb.reshape(2, 4, 4, 128, NQC, NQ).transpose(4, 1, 3, 2, 0, 5)
        cm["EB"] = np.ascontiguousarray(
            eb7.reshape(NQC * 4 * 128, 2 * 2048).astype(ml_dtypes.bfloat16))
        in_maps.append(cm)
    return in_maps


def kernel(x, lag, wq, bq, wk, bk, wv, bv, wo, bo, lag_bias):
    nc = _get_nc()
    in_maps = _host_prep(x, lag, wq, bq, wk, bk, wv, bv, wo, bo, lag_bias)
    kwargs = {}
    if TRACE:
        kwargs = dict(trace=True, tmpdir=TRACE_DIR)
    res = run_bass_kernel_spmd(nc, in_maps, core_ids=list(range(N_CORES)),
                               **kwargs)
    if TRACE:
        print(f"HW exec time: {res.exec_time_ns} ns")
    total = res.results[0]["out"].astype(np.float32)
    for c in range(1, N_CORES):
        total += res.results[c]["out"].astype(np.float32)
    total += np.asarray(bo, dtype=np.float32)[None, :]
    return total.reshape(B, S, D)
